# revision 21
# baseline (speedup 1.0000x reference)
"""Causal single-head attention (B=4, S=4096, D=1024, H=128) on 8 NeuronCores.

Sharding: core c = (batch b = c//2, half h = c%2). Each core:
  - computes K^T [h, 4096] and V [4096, H] for its full batch row (replicated
    across the 2 cores of a batch),
  - handles 2048 query rows: 16 parity-interleaved 128-row subtiles
    (global subtile g = 8*r + 2*s + h for slot r in 0..3, s in 0..3),
  - slots have uniform causal k-tile limits [8, 16, 24, 32] so all 8 cores run
    the identical compiled program; causality is enforced with per-core mask
    DATA (host-precomputed 0/1 patterns, shared across slots) multiplied into
    P on the last 8 k-iters of each slot; only the two 128-col subtiles at the
    causal frontier are touched.

Mixed precision (validated vs fp32 reference, rel_err ~3.4e-3):
  - x cols 0:1024 + q-slot-0 columns arrive bf16; everything else fp8 e4m3
    (x pre-scaled x8, weights x64; 1/512 folded into the PSUM->SBUF copy)
  - K^T bricks 0..7, Q^T slot 0, V bricks 0..7 projected in bf16; all other
    projections fp8 DoubleRow (two 128-chunks contracted per matmul, 2x PE)
  - scores always bf16 (fp8 gains nothing at contraction depth 128); P and V
    fp8 for slots 1..3 (q >= 1024): PV runs DoubleRow over k-tile pairs, exp
    is one ACT op per pair over a 2-bank PSUM sT pair; slot 0 stays bf16
Engine balance: odd-kt denominator adds of slots 1,3 and their masks run on
GpSimd (Pool); even adds + slots 0,2 masks on DVE; exp/copies on ACT.
Denominator: dacc (bf16 partial sums per k-lane) contracted against a
sqrt(H)-valued ones vector, [128q,1] matmuls -> fp32 PSUM -> DVE reciprocal.
DMA: constants packed host-side into per-partition-contiguous tensors (2-6KB
descriptors instead of 4-256B), x in 2-chunk transfers, two HW queues (sync,
scalar) fed in mirrored need-time priority order.
"""

import numpy as np
import ml_dtypes
from contextlib import ExitStack

import concourse.bass as bass
import concourse.tile as tile
from concourse import bacc, mybir
from concourse.bass_utils import run_bass_kernel_spmd

B, S, D, H = 4, 4096, 1024, 128
P = 128
BF16 = mybir.dt.bfloat16
F32 = mybir.dt.float32
F8 = mybir.dt.float8e4
NPBF16 = ml_dtypes.bfloat16
NPF8 = ml_dtypes.float8_e4m3

QLOC = 2048          # query rows per core
NSLOT = 4            # slots per core
SLOT_W = 512         # q columns per slot
LIMITS = [8, 16, 24, 32]   # k-tile limit per slot (same for every core)
NKT = S // P         # 32 k tiles
DCH = D // P         # 8 contraction chunks
SX = 8.0             # host scale on x before fp8 cast
SW = 64.0            # host scale on W before fp8 cast (Q,K); V uses SW/8
QKS = 64.0 * SX      # Q/K PSUM arrives as QKS*(xW) on both paths
VS = 64.0            # V PSUM arrives as VS*(xW) on both paths
SCALE = 1.0 / (float(np.sqrt(H)) * QKS * QKS)   # pre-exp scale (+descale)
ONEVAL = VS * float(np.sqrt(H))     # denominator ones: folds sqrt(H) and 1/VS
NB16 = 1024          # x columns (and q rows) kept in bf16
DR = mybir.MatmulPerfMode.DoubleRow
WIX = {"wq": 0, "wk": 1, "wv": 2}
BIX = {"bq": 0, "bk": 1, "bv": 2}


def qglob_for_core(h):
    """Global query row indices (length QLOC) handled by core-half h, in local order."""
    idx = []
    for r in range(NSLOT):
        for s in range(4):
            g = 8 * r + 2 * s + h
            idx.append(np.arange(g * P, (g + 1) * P))
    return np.concatenate(idx)


def mask_patterns(h):
    """[P, 8, 2P] 0/1 causal patterns: entry u covers window columns
    [128*(u//2), 128*(u//2)+256) of a slot at mask k-iter u (kt = 8r+u)."""
    pat = np.zeros((P, 8, 2 * P), np.float32)
    p = np.arange(P)[:, None]
    for u in range(8):
        qq = np.arange(2 * P) + P * (u // 2)
        qpos = (2 * (qq // P) + h) * P + qq % P
        pat[:, u, :] = (qpos[None, :] >= u * P + p)
    return pat


def build_nc():
    nc = bacc.Bacc(None, target_bir_lowering=False, debug=False, num_devices=8)

    xt16 = nc.dram_tensor("xt16", [D, NB16], BF16, kind="ExternalInput").ap()
    xt8 = nc.dram_tensor("xt8", [D, S - NB16], F8, kind="ExternalInput").ap()
    xqt16 = nc.dram_tensor("xqt16", [D, SLOT_W], BF16, kind="ExternalInput").ap()
    xqt8 = nc.dram_tensor("xqt8", [D, QLOC - SLOT_W], F8, kind="ExternalInput").ap()
    wp16 = nc.dram_tensor("wp16", [P, 3, DCH, H], BF16, kind="ExternalInput").ap()
    wp8 = nc.dram_tensor("wp8", [P, 3, DCH, H], F8, kind="ExternalInput").ap()
    bpk = nc.dram_tensor("bpk", [P, 3], F32, kind="ExternalInput").ap()
    idon = nc.dram_tensor("idon", [P, P + 1], BF16, kind="ExternalInput").ap()
    mp16 = nc.dram_tensor("mp16", [P, 8, 2 * P], BF16, kind="ExternalInput").ap()
    mp8 = nc.dram_tensor("mp8", [P, 8, 2 * P], F8, kind="ExternalInput").ap()
    out = nc.dram_tensor("out", [QLOC, H], F32, kind="ExternalOutput").ap()

    Ident = mybir.ActivationFunctionType.Identity
    Copy = mybir.ActivationFunctionType.Copy
    Exp = mybir.ActivationFunctionType.Exp

    with tile.TileContext(nc) as tc, ExitStack() as ctx:
        consts = ctx.enter_context(tc.tile_pool(name="consts", bufs=1))
        persist = ctx.enter_context(tc.tile_pool(name="persist", bufs=1))

        # ---- SBUF destinations (packed)
        wp16_sb = consts.tile([P, 3, DCH, H], BF16, tag="wp16")
        wp8_sb = consts.tile([P, 3, DCH, H], F8, tag="wp8")
        bpk_sb = consts.tile([P, 3], F32, tag="bpk")
        idon_sb = consts.tile([P, P + 1], BF16, tag="idon")
        mp16_sb = consts.tile([P, 8, 2 * P], BF16, tag="mp16")
        mp8_sb = consts.tile([P, 8, 2 * P], F8, tag="mp8")
        identb_v = idon_sb[:, 0:P]
        ones_v = idon_sb[:, P:P + 1]

        def bias(nm):
            return bpk_sb[:, BIX[nm]:BIX[nm] + 1]

        kT = persist.tile([P, S], BF16, tag="kT")             # K^T [h, s]
        vN16 = persist.tile([P, 8, H], BF16, tag="vN16")      # V natural, bricks 0..7
        vN8 = persist.tile([P, NKT, H], F8, tag="vN8")        # V natural fp8, all bricks
        qT = persist.tile([P, QLOC], BF16, tag="qT")          # Q^T [h, q_local]
        xt16_sb = persist.tile([P, DCH, NB16], BF16, tag="xt16_sb")
        xt8_sb = persist.tile([P, DCH, S - NB16], F8, tag="xt8_sb")
        xqt16_sb = persist.tile([P, DCH, SLOT_W], BF16, tag="xqt16_sb")
        xqt8_sb = persist.tile([P, DCH, QLOC - SLOT_W], F8, tag="xqt8_sb")

        # ---- DMA: chunk-pair transfers, two queues in mirrored priority order
        def load_pair(eng, dst, src, jp):
            eng.dma_start(
                out=dst[:, 2 * jp:2 * jp + 2, :],
                in_=src[2 * jp * P:(2 * jp + 2) * P, :].rearrange(
                    "(c p) s -> p c s", p=P),
            )

        # group 1: first K/V projection critical path
        nc.sync.dma_start(out=bpk_sb[:], in_=bpk)
        nc.sync.dma_start(out=wp16_sb[:], in_=wp16)
        load_pair(nc.sync, xt16_sb, xt16, 0)
        load_pair(nc.scalar, xt16_sb, xt16, 2)
        load_pair(nc.sync, xt16_sb, xt16, 1)
        load_pair(nc.scalar, xt16_sb, xt16, 3)
        # group 2: Q slot 0 + V transpose ident + early (slot-0) mask patterns
        nc.sync.dma_start(out=idon_sb[:], in_=idon)
        load_pair(nc.scalar, xqt16_sb, xqt16, 0)
        load_pair(nc.scalar, xqt16_sb, xqt16, 1)
        nc.sync.dma_start(out=mp16_sb[:], in_=mp16)
        load_pair(nc.scalar, xqt16_sb, xqt16, 2)
        load_pair(nc.scalar, xqt16_sb, xqt16, 3)
        # group 3: fp8 stripes 2,3 (kT/vN bricks 8..15, pass-1 tail)
        nc.sync.dma_start(out=wp8_sb[:], in_=wp8)
        load_pair(nc.sync, xt8_sb, xt8, 0)
        load_pair(nc.scalar, xt8_sb, xt8, 1)
        nc.sync.dma_start(out=mp8_sb[:], in_=mp8)
        # group 4: Q slots 1..3
        load_pair(nc.sync, xqt8_sb, xqt8, 0)
        load_pair(nc.scalar, xqt8_sb, xqt8, 2)
        load_pair(nc.sync, xqt8_sb, xqt8, 1)
        load_pair(nc.scalar, xqt8_sb, xqt8, 3)
        # group 5: fp8 stripes 4..7 (pass 2)
        load_pair(nc.sync, xt8_sb, xt8, 2)
        load_pair(nc.scalar, xt8_sb, xt8, 3)

        # PSUM budget (8 banks): mm x2 + sT-pair x2(4 banks) + oT-pair (2) = 8
        with tc.tile_pool(name="stg", bufs=3) as stg, \
             tc.tile_pool(name="mm", bufs=2, space="PSUM") as psA, \
             tc.tile_pool(name="psS", bufs=2, space="PSUM") as psS, \
             tc.tile_pool(name="psO", bufs=1, space="PSUM") as psO, \
             tc.tile_pool(name="pp", bufs=6) as pp, \
             tc.tile_pool(name="pp8", bufs=6) as pp8, \
             tc.tile_pool(name="acc", bufs=1) as accp, \
             tc.tile_pool(name="epi", bufs=4) as epi:

            def project16(src_sb, srs, wname):
                """bf16 weight-stationary projection of 512-col stripes of src."""
                pss = [psA.tile([P, SLOT_W], F32, tag="mm512", name=f"p16_{i}")
                       for i in range(len(srs))]
                for j in range(DCH):
                    for i, sr in enumerate(srs):
                        nc.tensor.matmul(
                            pss[i][:], lhsT=wp16_sb[:, WIX[wname], j, :],
                            rhs=src_sb[:, j, sr * SLOT_W:(sr + 1) * SLOT_W],
                            start=(j == 0), stop=(j == DCH - 1),
                        )
                return pss

            def project8(src_sb, srs, wname):
                """fp8 DoubleRow projection (chunk pairs) of 512-col stripes."""
                pss = [psA.tile([P, SLOT_W], F32, tag="mm512", name=f"p8_{i}")
                       for i in range(len(srs))]
                for j in range(DCH // 2):
                    for i, sr in enumerate(srs):
                        nc.tensor.matmul(
                            pss[i][:], lhsT=wp8_sb[:, WIX[wname], 2 * j:2 * j + 2, :],
                            rhs=src_sb[:, 2 * j:2 * j + 2, sr * SLOT_W:(sr + 1) * SLOT_W],
                            start=(j == 0), stop=(j == DCH // 2 - 1), perf_mode=DR,
                        )
                return pss

            def transpose_bricks(vTs, sr, dst):
                """PE-transpose a bf16 512-col V^T stripe into 4 natural bricks
                of dst (fp8 dst converts on the DVE copy)."""
                pst = psA.tile([P, SLOT_W], BF16, tag="mm512", name="vtr")
                for t_ in range(4):
                    nc.tensor.matmul(
                        pst[:, t_ * P:(t_ + 1) * P], lhsT=vTs[:, t_ * P:(t_ + 1) * P],
                        rhs=identb_v, is_transpose=True, skip_group_check=True,
                    )
                if dst is vN8:
                    nc.scalar.activation(dst[:, sr * 4:(sr + 1) * 4, :], pst[:],
                                         Copy)
                else:
                    nc.vector.tensor_copy(dst[:, sr * 4:(sr + 1) * 4, :], pst[:])

            def kv_common(kps, vps, srs):
                """PSUM->SBUF: K^T via DVE bias-add (frees ACT for exp);
                V^T via ACT bias-add, then PE transpose."""
                for ps, sr in zip(kps, srs):
                    nc.vector.tensor_scalar_add(
                        kT[:, sr * SLOT_W:(sr + 1) * SLOT_W], ps[:], bias("bk"))
                for ps, sr in zip(vps, srs):
                    vTs = stg.tile([P, SLOT_W], BF16, tag="vT")
                    nc.scalar.activation(vTs[:], ps[:], Ident, bias=bias("bv"),
                                         scale=1.0)
                    transpose_bricks(vTs, sr, vN16 if sr < 2 else vN8)

            def kv_stripes16():
                kv_common(project16(xt16_sb, (0, 1), "wk"),
                          project16(xt16_sb, (0, 1), "wv"), (0, 1))
                nc.vector.tensor_copy(vN8[:, 0:8, :], vN16[:])

            def kv_stripes8(*srs):
                lsrs = [sr - 2 for sr in srs]   # xt8 local stripe index
                kv_common(project8(xt8_sb, lsrs, "wk"),
                          project8(xt8_sb, lsrs, "wv"), srs)

            def q_slot0():
                (ps,) = project16(xqt16_sb, (0,), "wq")
                nc.vector.tensor_scalar_add(qT[:, 0:SLOT_W], ps[:], bias("bq"))

            def q_slots8(*qrs):
                lqs = [qr - 1 for qr in qrs]   # xqt8 local stripe index
                for ps, qr in zip(project8(xqt8_sb, lqs, "wq"), qrs):
                    nc.vector.tensor_scalar_add(
                        qT[:, qr * SLOT_W:(qr + 1) * SLOT_W], ps[:], bias("bq"))

            def attention_pass(slots):
                """kt-pair-outer attention over a pair of slots (shared K/V)."""
                Ls = {r: LIMITS[r] for r in slots}
                Tmax = max(Ls.values()) // 2
                oT = psO.tile([P, len(slots), SLOT_W], F32, tag="oT")
                dacc = {r: accp.tile([P, SLOT_W], BF16, tag=f"dacc{r}", name=f"dacc{r}")
                        for r in slots}

                def c0_of(r, kt):
                    # first column (h-safe) any core's subtile can still attend
                    # at this k-brick; earlier columns are masked for both
                    # halves and skipped (the pair uses the even-kt value; the
                    # mask pattern zeroes the stale subtile)
                    return P * max(0, (kt - 8 * r) // 2)

                def dacc_upd(r, src_ap, c0, first, pool=False):
                    if first:
                        nc.vector.tensor_copy(dacc[r][:], src_ap)
                    else:
                        eng = nc.gpsimd if pool else nc.vector
                        eng.tensor_add(dacc[r][:, c0:], dacc[r][:, c0:], src_ap)

                def mask_op(r, kt, dst_ap, c0):
                    # p *= 0/1 pattern; only the two subtiles at the causal
                    # frontier are affected. Pool engine for slots 1,3.
                    u = kt - 8 * r
                    w = min(2 * P, SLOT_W - c0)
                    mp = mp16_sb if r == 0 else mp8_sb
                    eng = nc.vector if r == 0 else nc.gpsimd
                    eng.tensor_mul(dst_ap[:, :w], dst_ap[:, :w], mp[:, u, :w])

                def score_pair(t):
                    outs = {}
                    for r in slots:
                        if 2 * t >= Ls[r]:
                            continue
                        c0 = c0_of(r, 2 * t)
                        qsl = slice(r * SLOT_W + c0, (r + 1) * SLOT_W)
                        sTp = psS.tile([P, 2, SLOT_W], F32, tag="sT")
                        for i in range(2):
                            nc.tensor.matmul(
                                sTp[:, i, c0:],
                                lhsT=kT[:, (2 * t + i) * P:(2 * t + i + 1) * P],
                                rhs=qT[:, qsl], start=True, stop=True,
                            )
                        masked = 2 * t >= Ls[r] - 8
                        if r == 0:
                            pTs = []
                            for i in range(2):
                                pT = pp.tile([P, SLOT_W], BF16, tag="pT")
                                nc.scalar.activation(pT[:, c0:], sTp[:, i, c0:],
                                                     Exp, scale=SCALE)
                                if masked:
                                    mask_op(r, 2 * t + i, pT[:, c0:], c0)
                                dacc_upd(r, pT[:, c0:], c0, first=(t == 0 and i == 0))
                                pTs.append(pT)
                            outs[r] = (pTs, c0)
                        else:
                            pT8 = pp8.tile([P, 2, SLOT_W], F8, tag="pT8")
                            nc.scalar.activation(pT8[:, :, c0:], sTp[:, :, c0:],
                                                 Exp, scale=SCALE)
                            for i in range(2):
                                if masked:
                                    mask_op(r, 2 * t + i, pT8[:, i, c0:], c0)
                                dacc_upd(r, pT8[:, i, c0:], c0,
                                         first=(t == 0 and i == 0),
                                         pool=(i == 1))
                            outs[r] = (pT8, c0)
                    return outs

                def accum_pair(t, outs):
                    for i_s, r in enumerate(slots):
                        if r not in outs:
                            continue
                        buf, c0 = outs[r]
                        if r == 0:
                            for i in range(2):
                                nc.tensor.matmul(
                                    oT[:, i_s, c0:], lhsT=vN16[:, 2 * t + i, :],
                                    rhs=buf[i][:, c0:],
                                    start=(t == 0 and i == 0),
                                    stop=(2 * t + i == Ls[r] - 1),
                                )
                        else:
                            nc.tensor.matmul(
                                oT[:, i_s, c0:], lhsT=vN8[:, 2 * t:2 * t + 2, :],
                                rhs=buf[:, :, c0:],
                                start=(t == 0), stop=(2 * t + 1 == Ls[r] - 1),
                                perf_mode=DR,
                            )

                def epilogue(i_s, r):
                    """O = transpose(O^T) / (d * sqrt(H)) for one slot."""
                    d_ps = psA.tile([P, 4], F32, tag="mm512", name="dmm")
                    for s_ in range(4):
                        nc.tensor.matmul(
                            d_ps[:, s_:s_ + 1],
                            lhsT=dacc[r][:, s_ * P:(s_ + 1) * P], rhs=ones_v,
                            start=(s_ == 0), stop=(s_ == 3), skip_group_check=True,
                        )
                    oTs = epi.tile([P, SLOT_W], BF16, tag="oTs")
                    nc.scalar.activation(oTs[:], oT[:, i_s, :], Copy)
                    rec = epi.tile([P, 4], F32, tag="rec")
                    nc.vector.reciprocal(rec[:], d_ps[:])
                    obr = psA.tile([P, SLOT_W], BF16, tag="mm512", name="obr")
                    for s_ in range(4):
                        nc.tensor.matmul(
                            obr[:, s_ * P:(s_ + 1) * P], lhsT=oTs[:, s_ * P:(s_ + 1) * P],
                            rhs=identb_v, is_transpose=True, skip_group_check=True,
                        )
                    ofin = epi.tile([P, SLOT_W], F32, tag="ofin")
                    for s_ in range(4):
                        nc.vector.tensor_scalar_mul(
                            ofin[:, s_ * P:(s_ + 1) * P], obr[:, s_ * P:(s_ + 1) * P],
                            rec[:, s_:s_ + 1],
                        )
                    nc.sync.dma_start(
                        out=out[r * SLOT_W:(r + 1) * SLOT_W, :].rearrange(
                            "(s p) h -> p s h", p=P
                        ),
                        in_=ofin[:].rearrange("p (s h) -> p s h", s=4),
                    )

                prev = score_pair(0)
                for t in range(1, Tmax):
                    cur = score_pair(t)
                    accum_pair(t - 1, prev)
                    prev = cur
                    # emit the shorter slot's epilogue as soon as it stops
                    for i_s, r in enumerate(slots):
                        if Ls[r] == 2 * t:
                            epilogue(i_s, r)
                accum_pair(Tmax - 1, prev)
                for i_s, r in enumerate(slots):
                    if Ls[r] == 2 * Tmax:
                        epilogue(i_s, r)

            # emission: all projections first (their PSUM slot allocations must
            # not queue behind pass epilogues), then the attention passes
            kv_stripes16()
            q_slot0()
            kv_stripes8(2, 3)
            q_slots8(1)
            kv_stripes8(4, 5)
            kv_stripes8(6, 7)
            q_slots8(2, 3)
            attention_pass((0, 1))
            attention_pass((2, 3))

    nc.compile()
    return nc


_NC_CACHE = None


def _get_nc():
    global _NC_CACHE
    if _NC_CACHE is None:
        _NC_CACHE = build_nc()
    return _NC_CACHE


def make_in_maps(inputs):
    x = np.asarray(inputs["x"], np.float32)
    Ws = {nm: np.asarray(inputs[Wnm], np.float32)
          for nm, Wnm in (("wq", "Wq"), ("wk", "Wk"), ("wv", "Wv"))}
    bs = {nm: np.asarray(inputs[bnm], np.float32)
          for nm, bnm in (("bq", "bq"), ("bk", "bk"), ("bv", "bv"))}

    # packed weights: wp[p, i, c, h] = W_i[c*128+p, h]
    wp16 = np.zeros((P, 3, DCH, H), NPBF16)
    wp8 = np.zeros((P, 3, DCH, H), NPF8)
    for nm, W in Ws.items():
        s16 = QKS if nm in ("wq", "wk") else VS
        Wc = W.reshape(DCH, P, H).transpose(1, 0, 2)   # [p, c, h]
        wp16[:, WIX[nm]] = (Wc * s16).astype(NPBF16)
        wp8[:, WIX[nm]] = (Wc * (s16 / SX)).astype(NPF8)
    bpk = np.zeros((P, 3), np.float32)
    for nm, b in bs.items():
        bpk[:, BIX[nm]] = b * (QKS if nm in ("bq", "bk") else VS)
    idon = np.concatenate(
        [np.eye(P, dtype=NPBF16), np.full((P, 1), ONEVAL, NPBF16)], axis=1)

    common = dict(wp16=wp16, wp8=wp8, bpk=bpk, idon=idon)
    in_maps = []
    xT = np.ascontiguousarray(x.transpose(0, 2, 1))        # [B, D, S] fp32
    xT16 = xT.astype(NPBF16)
    xT8 = (xT * SX).astype(NPF8)
    mps = {h: mask_patterns(h) for h in (0, 1)}
    for c in range(8):
        b, hh = c // 2, c % 2
        qg = qglob_for_core(hh)
        m = dict(common)
        m["xt16"] = xT16[b][:, :NB16]
        m["xt8"] = np.ascontiguousarray(xT8[b][:, NB16:])
        m["xqt16"] = np.ascontiguousarray(xT16[b][:, qg[:SLOT_W]])
        m["xqt8"] = np.ascontiguousarray(xT8[b][:, qg[SLOT_W:]])
        m["mp16"] = mps[hh].astype(NPBF16)
        m["mp8"] = mps[hh].astype(NPF8)
        in_maps.append(m)
    return in_maps


def assemble_out(results):
    out = np.zeros((1, B, S, H), np.float32)
    for c in range(8):
        b, hh = c // 2, c % 2
        qg = qglob_for_core(hh)
        out[0, b, qg, :] = results[c]["out"]
    return out


def kernel(**inputs) -> np.ndarray:
    nc = _get_nc()
    in_maps = make_in_maps(inputs)
    res = run_bass_kernel_spmd(nc, in_maps, list(range(8)))
    return assemble_out(res.results)


# revision 22
# speedup vs baseline: 1.0659x; 1.0659x over previous
"""Causal single-head attention (B=4, S=4096, D=1024, H=128) on 8 NeuronCores.

Sharding: core c = (batch b = c//2, half h = c%2). Each core:
  - computes K^T [h, 4096] and V [4096, H] for its full batch row (replicated
    across the 2 cores of a batch),
  - handles 2048 query rows: 16 parity-interleaved 128-row subtiles
    (global subtile g = 8*r + 2*s + h for slot r in 0..3, s in 0..3),
  - slots have uniform causal k-tile limits [8, 16, 24, 32] so all 8 cores run
    the identical compiled program; causality is enforced with per-core mask
    DATA (host-precomputed 0/1 patterns, shared across slots) multiplied into
    P on the last 8 k-iters of each slot; only the two 128-col subtiles at the
    causal frontier are touched.

Mixed precision (validated vs fp32 reference, rel_err ~3.4e-3):
  - x cols 0:1024 + q-slot-0 columns arrive bf16; everything else fp8 e4m3
    (x pre-scaled x8, weights x64; 1/512 folded into the PSUM->SBUF copy)
  - K^T bricks 0..7, Q^T slot 0, V bricks 0..7 projected in bf16; all other
    projections fp8 DoubleRow (two 128-chunks contracted per matmul, 2x PE)
  - scores always bf16 (fp8 gains nothing at contraction depth 128); P and V
    fp8 for slots 1..3 (q >= 1024): PV runs DoubleRow over k-tile pairs, exp
    is one ACT op per pair over a 2-bank PSUM sT pair; slot 0 stays bf16
Engine balance: odd-kt denominator adds of slots 1,3 and their masks run on
GpSimd (Pool); even adds + slots 0,2 masks on DVE; exp/copies on ACT.
Denominator: dacc (bf16 partial sums per k-lane) contracted against a
sqrt(H)-valued ones vector, [128q,1] matmuls -> fp32 PSUM -> DVE reciprocal.
DMA: constants packed host-side into per-partition-contiguous tensors (2-6KB
descriptors instead of 4-256B), x in 2-chunk transfers, two HW queues (sync,
scalar) fed in mirrored need-time priority order.
"""

import numpy as np
import ml_dtypes
from contextlib import ExitStack

import concourse.bass as bass
import concourse.tile as tile
from concourse import bacc, mybir
from concourse.bass_utils import run_bass_kernel_spmd

B, S, D, H = 4, 4096, 1024, 128
P = 128
BF16 = mybir.dt.bfloat16
F32 = mybir.dt.float32
F8 = mybir.dt.float8e4
NPBF16 = ml_dtypes.bfloat16
NPF8 = ml_dtypes.float8_e4m3

QLOC = 2048          # query rows per core
NSLOT = 4            # slots per core
SLOT_W = 512         # q columns per slot
LIMITS = [8, 16, 24, 32]   # k-tile limit per slot (same for every core)
NKT = S // P         # 32 k tiles
DCH = D // P         # 8 contraction chunks
SX = 8.0             # host scale on x before fp8 cast
SW = 64.0            # host scale on W before fp8 cast (Q,K); V uses SW/8
QKS = 64.0 * SX      # Q/K PSUM arrives as QKS*(xW) on both paths
VS = 64.0            # V PSUM arrives as VS*(xW) on both paths
SCALE = 1.0 / (float(np.sqrt(H)) * QKS * QKS)   # pre-exp scale (+descale)
ONEVAL = VS * float(np.sqrt(H))     # denominator ones: folds sqrt(H) and 1/VS
NB16 = 1024          # x columns (and q rows) kept in bf16
DR = mybir.MatmulPerfMode.DoubleRow
WIX = {"wq": 0, "wk": 1, "wv": 2}
BIX = {"bq": 0, "bk": 1, "bv": 2}


def qglob_for_core(h):
    """Global query row indices (length QLOC) handled by core-half h, in local order."""
    idx = []
    for r in range(NSLOT):
        for s in range(4):
            g = 8 * r + 2 * s + h
            idx.append(np.arange(g * P, (g + 1) * P))
    return np.concatenate(idx)


def mask_patterns(h):
    """[P, 8, 2P] 0/1 causal patterns: entry u covers window columns
    [128*(u//2), 128*(u//2)+256) of a slot at mask k-iter u (kt = 8r+u)."""
    pat = np.zeros((P, 8, 2 * P), np.float32)
    p = np.arange(P)[:, None]
    for u in range(8):
        qq = np.arange(2 * P) + P * (u // 2)
        qpos = (2 * (qq // P) + h) * P + qq % P
        pat[:, u, :] = (qpos[None, :] >= u * P + p)
    return pat


def build_nc():
    nc = bacc.Bacc(None, target_bir_lowering=False, debug=False, num_devices=8)

    xt16 = nc.dram_tensor("xt16", [D, NB16], BF16, kind="ExternalInput").ap()
    xt8 = nc.dram_tensor("xt8", [D, S - NB16], F8, kind="ExternalInput").ap()
    xqt16 = nc.dram_tensor("xqt16", [D, SLOT_W], BF16, kind="ExternalInput").ap()
    xqt8 = nc.dram_tensor("xqt8", [D, QLOC - SLOT_W], F8, kind="ExternalInput").ap()
    wp16 = nc.dram_tensor("wp16", [P, 3, DCH, H], BF16, kind="ExternalInput").ap()
    wp8 = nc.dram_tensor("wp8", [P, 3, DCH, H], F8, kind="ExternalInput").ap()
    bpk = nc.dram_tensor("bpk", [P, 3], F32, kind="ExternalInput").ap()
    idon = nc.dram_tensor("idon", [P, P + 1], BF16, kind="ExternalInput").ap()
    mp16 = nc.dram_tensor("mp16", [P, 8, 2 * P], BF16, kind="ExternalInput").ap()
    mp8 = nc.dram_tensor("mp8", [P, 8, 2 * P], F8, kind="ExternalInput").ap()
    out = nc.dram_tensor("out", [QLOC, H], F32, kind="ExternalOutput").ap()

    Ident = mybir.ActivationFunctionType.Identity
    Copy = mybir.ActivationFunctionType.Copy
    Exp = mybir.ActivationFunctionType.Exp

    with tile.TileContext(nc) as tc, ExitStack() as ctx:
        consts = ctx.enter_context(tc.tile_pool(name="consts", bufs=1))
        persist = ctx.enter_context(tc.tile_pool(name="persist", bufs=1))

        # ---- SBUF destinations (packed)
        wp16_sb = consts.tile([P, 3, DCH, H], BF16, tag="wp16")
        wp8_sb = consts.tile([P, 3, DCH, H], F8, tag="wp8")
        bpk_sb = consts.tile([P, 3], F32, tag="bpk")
        idon_sb = consts.tile([P, P + 1], BF16, tag="idon")
        mp16_sb = consts.tile([P, 8, 2 * P], BF16, tag="mp16")
        mp8_sb = consts.tile([P, 8, 2 * P], F8, tag="mp8")
        identb_v = idon_sb[:, 0:P]
        ones_v = idon_sb[:, P:P + 1]

        def bias(nm):
            return bpk_sb[:, BIX[nm]:BIX[nm] + 1]

        kT = persist.tile([P, S], BF16, tag="kT")             # K^T [h, s]
        vN16 = persist.tile([P, 8, H], BF16, tag="vN16")      # V natural, bricks 0..7
        vN8 = persist.tile([P, NKT, H], F8, tag="vN8")        # V natural fp8, all bricks
        qT = persist.tile([P, QLOC], BF16, tag="qT")          # Q^T [h, q_local]
        xt16_sb = persist.tile([P, DCH, NB16], BF16, tag="xt16_sb")
        xt8_sb = persist.tile([P, DCH, S - NB16], F8, tag="xt8_sb")
        xqt16_sb = persist.tile([P, DCH, SLOT_W], BF16, tag="xqt16_sb")
        xqt8_sb = persist.tile([P, DCH, QLOC - SLOT_W], F8, tag="xqt8_sb")

        # ---- DMA: chunk-pair transfers, two queues in mirrored priority order
        def load_pair(eng, dst, src, jp):
            eng.dma_start(
                out=dst[:, 2 * jp:2 * jp + 2, :],
                in_=src[2 * jp * P:(2 * jp + 2) * P, :].rearrange(
                    "(c p) s -> p c s", p=P),
            )

        # group 1: first K/V projection critical path
        nc.sync.dma_start(out=bpk_sb[:], in_=bpk)
        nc.sync.dma_start(out=wp16_sb[:], in_=wp16)
        load_pair(nc.sync, xt16_sb, xt16, 0)
        load_pair(nc.scalar, xt16_sb, xt16, 2)
        load_pair(nc.sync, xt16_sb, xt16, 1)
        load_pair(nc.scalar, xt16_sb, xt16, 3)
        # group 2: Q slot 0 + V transpose ident + early (slot-0) mask patterns
        nc.sync.dma_start(out=idon_sb[:], in_=idon)
        load_pair(nc.scalar, xqt16_sb, xqt16, 0)
        load_pair(nc.scalar, xqt16_sb, xqt16, 1)
        nc.sync.dma_start(out=mp16_sb[:], in_=mp16)
        load_pair(nc.scalar, xqt16_sb, xqt16, 2)
        load_pair(nc.scalar, xqt16_sb, xqt16, 3)
        # group 3: fp8 stripes 2,3 (kT/vN bricks 8..15, pass-1 tail)
        nc.sync.dma_start(out=wp8_sb[:], in_=wp8)
        load_pair(nc.sync, xt8_sb, xt8, 0)
        load_pair(nc.scalar, xt8_sb, xt8, 1)
        nc.sync.dma_start(out=mp8_sb[:], in_=mp8)
        # group 4: Q slots 1..3
        load_pair(nc.sync, xqt8_sb, xqt8, 0)
        load_pair(nc.scalar, xqt8_sb, xqt8, 2)
        load_pair(nc.sync, xqt8_sb, xqt8, 1)
        load_pair(nc.scalar, xqt8_sb, xqt8, 3)
        # group 5: fp8 stripes 4..7 (pass 2)
        load_pair(nc.sync, xt8_sb, xt8, 2)
        load_pair(nc.scalar, xt8_sb, xt8, 3)

        # PSUM budget (8 banks): mm x2 + sT-pair x2(4 banks) + oT-pair (2) = 8
        with tc.tile_pool(name="stg", bufs=3) as stg, \
             tc.tile_pool(name="mm", bufs=2, space="PSUM") as psA, \
             tc.tile_pool(name="psS", bufs=2, space="PSUM") as psS, \
             tc.tile_pool(name="psO", bufs=1, space="PSUM") as psO, \
             tc.tile_pool(name="pp", bufs=6) as pp, \
             tc.tile_pool(name="pp8", bufs=6) as pp8, \
             tc.tile_pool(name="acc", bufs=1) as accp, \
             tc.tile_pool(name="epi", bufs=4) as epi:

            def project16(src_sb, srs, wname):
                """bf16 weight-stationary projection of 512-col stripes of src."""
                pss = [psA.tile([P, SLOT_W], F32, tag="mm512", name=f"p16_{i}")
                       for i in range(len(srs))]
                for j in range(DCH):
                    for i, sr in enumerate(srs):
                        nc.tensor.matmul(
                            pss[i][:], lhsT=wp16_sb[:, WIX[wname], j, :],
                            rhs=src_sb[:, j, sr * SLOT_W:(sr + 1) * SLOT_W],
                            start=(j == 0), stop=(j == DCH - 1),
                        )
                return pss

            def project8(src_sb, srs, wname):
                """fp8 DoubleRow projection (chunk pairs) of 512-col stripes."""
                pss = [psA.tile([P, SLOT_W], F32, tag="mm512", name=f"p8_{i}")
                       for i in range(len(srs))]
                for j in range(DCH // 2):
                    for i, sr in enumerate(srs):
                        nc.tensor.matmul(
                            pss[i][:], lhsT=wp8_sb[:, WIX[wname], 2 * j:2 * j + 2, :],
                            rhs=src_sb[:, 2 * j:2 * j + 2, sr * SLOT_W:(sr + 1) * SLOT_W],
                            start=(j == 0), stop=(j == DCH // 2 - 1), perf_mode=DR,
                        )
                return pss

            def transpose_bricks(vTs, sr, dst):
                """PE-transpose a bf16 512-col V^T stripe into 4 natural bricks
                of dst (fp8 dst converts on the DVE copy)."""
                pst = psA.tile([P, SLOT_W], BF16, tag="mm512", name="vtr")
                for t_ in range(4):
                    nc.tensor.matmul(
                        pst[:, t_ * P:(t_ + 1) * P], lhsT=vTs[:, t_ * P:(t_ + 1) * P],
                        rhs=identb_v, is_transpose=True, skip_group_check=True,
                    )
                if dst is vN8:
                    nc.scalar.activation(dst[:, sr * 4:(sr + 1) * 4, :], pst[:],
                                         Copy)
                else:
                    nc.vector.tensor_copy(dst[:, sr * 4:(sr + 1) * 4, :], pst[:])

            def kv_common(kps, vps, srs):
                """PSUM->SBUF: K^T via DVE bias-add (frees ACT for exp);
                V^T via ACT bias-add, then PE transpose."""
                for ps, sr in zip(kps, srs):
                    nc.vector.tensor_scalar_add(
                        kT[:, sr * SLOT_W:(sr + 1) * SLOT_W], ps[:], bias("bk"))
                for ps, sr in zip(vps, srs):
                    vTs = stg.tile([P, SLOT_W], BF16, tag="vT")
                    nc.scalar.activation(vTs[:], ps[:], Ident, bias=bias("bv"),
                                         scale=1.0)
                    transpose_bricks(vTs, sr, vN16 if sr < 2 else vN8)

            def kv_stripes16():
                kv_common(project16(xt16_sb, (0, 1), "wk"),
                          project16(xt16_sb, (0, 1), "wv"), (0, 1))
                nc.vector.tensor_copy(vN8[:, 0:8, :], vN16[:])

            def kv_stripes8(*srs):
                lsrs = [sr - 2 for sr in srs]   # xt8 local stripe index
                kv_common(project8(xt8_sb, lsrs, "wk"),
                          project8(xt8_sb, lsrs, "wv"), srs)

            def q_slot0():
                (ps,) = project16(xqt16_sb, (0,), "wq")
                nc.vector.tensor_scalar_add(qT[:, 0:SLOT_W], ps[:], bias("bq"))

            def q_slots8(*qrs):
                lqs = [qr - 1 for qr in qrs]   # xqt8 local stripe index
                for ps, qr in zip(project8(xqt8_sb, lqs, "wq"), qrs):
                    nc.vector.tensor_scalar_add(
                        qT[:, qr * SLOT_W:(qr + 1) * SLOT_W], ps[:], bias("bq"))

            def attention_pass(slots):
                """kt-pair-outer attention over a pair of slots (shared K/V)."""
                Ls = {r: LIMITS[r] for r in slots}
                Tmax = max(Ls.values()) // 2
                oT = psO.tile([P, len(slots), SLOT_W], F32, tag="oT")
                dacc = {r: accp.tile([P, SLOT_W], BF16, tag=f"dacc{r}", name=f"dacc{r}")
                        for r in slots}

                def c0_of(r, kt):
                    # first column (h-safe) any core's subtile can still attend
                    # at this k-brick; earlier columns are masked for both
                    # halves and skipped (the pair uses the even-kt value; the
                    # mask pattern zeroes the stale subtile)
                    return P * max(0, (kt - 8 * r) // 2)

                def dacc_upd(r, src_ap, c0, first, pool=False):
                    if first:
                        nc.vector.tensor_copy(dacc[r][:], src_ap)
                    else:
                        eng = nc.gpsimd if pool else nc.vector
                        eng.tensor_add(dacc[r][:, c0:], dacc[r][:, c0:], src_ap)

                def mask_op(r, kt, dst_ap, c0):
                    # p *= 0/1 pattern; only the two subtiles at the causal
                    # frontier are affected. Pool engine for slots 1,3.
                    u = kt - 8 * r
                    w = min(2 * P, SLOT_W - c0)
                    mp = mp16_sb if r == 0 else mp8_sb
                    eng = nc.gpsimd if r in (1, 3) else nc.vector
                    eng.tensor_mul(dst_ap[:, :w], dst_ap[:, :w], mp[:, u, :w])

                def score_pair(t):
                    outs = {}
                    for r in slots:
                        if 2 * t >= Ls[r]:
                            continue
                        c0 = c0_of(r, 2 * t)
                        qsl = slice(r * SLOT_W + c0, (r + 1) * SLOT_W)
                        sTp = psS.tile([P, 2, SLOT_W], F32, tag="sT")
                        for i in range(2):
                            nc.tensor.matmul(
                                sTp[:, i, c0:],
                                lhsT=kT[:, (2 * t + i) * P:(2 * t + i + 1) * P],
                                rhs=qT[:, qsl], start=True, stop=True,
                            )
                        masked = 2 * t >= Ls[r] - 8
                        if r == 0:
                            pTs = []
                            for i in range(2):
                                pT = pp.tile([P, SLOT_W], BF16, tag="pT")
                                nc.scalar.activation(pT[:, c0:], sTp[:, i, c0:],
                                                     Exp, scale=SCALE)
                                if masked:
                                    mask_op(r, 2 * t + i, pT[:, c0:], c0)
                                dacc_upd(r, pT[:, c0:], c0, first=(t == 0 and i == 0))
                                pTs.append(pT)
                            outs[r] = (pTs, c0)
                        else:
                            pT8 = pp8.tile([P, 2, SLOT_W], F8, tag="pT8")
                            nc.scalar.activation(pT8[:, :, c0:], sTp[:, :, c0:],
                                                 Exp, scale=SCALE)
                            for i in range(2):
                                if masked:
                                    mask_op(r, 2 * t + i, pT8[:, i, c0:], c0)
                                dacc_upd(r, pT8[:, i, c0:], c0,
                                         first=(t == 0 and i == 0),
                                         pool=(i == 1 and r in (1, 3)))
                            outs[r] = (pT8, c0)
                    return outs

                def accum_pair(t, outs):
                    for i_s, r in enumerate(slots):
                        if r not in outs:
                            continue
                        buf, c0 = outs[r]
                        if r == 0:
                            for i in range(2):
                                nc.tensor.matmul(
                                    oT[:, i_s, c0:], lhsT=vN16[:, 2 * t + i, :],
                                    rhs=buf[i][:, c0:],
                                    start=(t == 0 and i == 0),
                                    stop=(2 * t + i == Ls[r] - 1),
                                )
                        else:
                            nc.tensor.matmul(
                                oT[:, i_s, c0:], lhsT=vN8[:, 2 * t:2 * t + 2, :],
                                rhs=buf[:, :, c0:],
                                start=(t == 0), stop=(2 * t + 1 == Ls[r] - 1),
                                perf_mode=DR,
                            )

                def epilogue(i_s, r):
                    """O = transpose(O^T) / (d * sqrt(H)) for one slot."""
                    d_ps = psA.tile([P, 4], F32, tag="mm512", name="dmm")
                    for s_ in range(4):
                        nc.tensor.matmul(
                            d_ps[:, s_:s_ + 1],
                            lhsT=dacc[r][:, s_ * P:(s_ + 1) * P], rhs=ones_v,
                            start=(s_ == 0), stop=(s_ == 3), skip_group_check=True,
                        )
                    oTs = epi.tile([P, SLOT_W], BF16, tag="oTs")
                    nc.scalar.activation(oTs[:], oT[:, i_s, :], Copy)
                    rec = epi.tile([P, 4], F32, tag="rec")
                    nc.vector.reciprocal(rec[:], d_ps[:])
                    obr = psA.tile([P, SLOT_W], BF16, tag="mm512", name="obr")
                    for s_ in range(4):
                        nc.tensor.matmul(
                            obr[:, s_ * P:(s_ + 1) * P], lhsT=oTs[:, s_ * P:(s_ + 1) * P],
                            rhs=identb_v, is_transpose=True, skip_group_check=True,
                        )
                    ofin = epi.tile([P, SLOT_W], F32, tag="ofin")
                    for s_ in range(4):
                        nc.vector.tensor_scalar_mul(
                            ofin[:, s_ * P:(s_ + 1) * P], obr[:, s_ * P:(s_ + 1) * P],
                            rec[:, s_:s_ + 1],
                        )
                    nc.sync.dma_start(
                        out=out[r * SLOT_W:(r + 1) * SLOT_W, :].rearrange(
                            "(s p) h -> p s h", p=P
                        ),
                        in_=ofin[:].rearrange("p (s h) -> p s h", s=4),
                    )

                prev = score_pair(0)
                for t in range(1, Tmax):
                    cur = score_pair(t)
                    accum_pair(t - 1, prev)
                    prev = cur
                    # emit the shorter slot's epilogue as soon as it stops
                    for i_s, r in enumerate(slots):
                        if Ls[r] == 2 * t:
                            epilogue(i_s, r)
                accum_pair(Tmax - 1, prev)
                for i_s, r in enumerate(slots):
                    if Ls[r] == 2 * Tmax:
                        epilogue(i_s, r)

            # emission: all projections first (their PSUM slot allocations must
            # not queue behind pass epilogues), then the attention passes
            kv_stripes16()
            q_slot0()
            kv_stripes8(2, 3)
            q_slots8(1)
            kv_stripes8(4, 5)
            kv_stripes8(6, 7)
            q_slots8(2, 3)
            attention_pass((0, 1))
            attention_pass((2, 3))

    nc.compile()
    return nc


_NC_CACHE = None


def _get_nc():
    global _NC_CACHE
    if _NC_CACHE is None:
        _NC_CACHE = build_nc()
    return _NC_CACHE


def make_in_maps(inputs):
    x = np.asarray(inputs["x"], np.float32)
    Ws = {nm: np.asarray(inputs[Wnm], np.float32)
          for nm, Wnm in (("wq", "Wq"), ("wk", "Wk"), ("wv", "Wv"))}
    bs = {nm: np.asarray(inputs[bnm], np.float32)
          for nm, bnm in (("bq", "bq"), ("bk", "bk"), ("bv", "bv"))}

    # packed weights: wp[p, i, c, h] = W_i[c*128+p, h]
    wp16 = np.zeros((P, 3, DCH, H), NPBF16)
    wp8 = np.zeros((P, 3, DCH, H), NPF8)
    for nm, W in Ws.items():
        s16 = QKS if nm in ("wq", "wk") else VS
        Wc = W.reshape(DCH, P, H).transpose(1, 0, 2)   # [p, c, h]
        wp16[:, WIX[nm]] = (Wc * s16).astype(NPBF16)
        wp8[:, WIX[nm]] = (Wc * (s16 / SX)).astype(NPF8)
    bpk = np.zeros((P, 3), np.float32)
    for nm, b in bs.items():
        bpk[:, BIX[nm]] = b * (QKS if nm in ("bq", "bk") else VS)
    idon = np.concatenate(
        [np.eye(P, dtype=NPBF16), np.full((P, 1), ONEVAL, NPBF16)], axis=1)

    common = dict(wp16=wp16, wp8=wp8, bpk=bpk, idon=idon)
    in_maps = []
    xT = np.ascontiguousarray(x.transpose(0, 2, 1))        # [B, D, S] fp32
    xT16 = xT.astype(NPBF16)
    xT8 = (xT * SX).astype(NPF8)
    mps = {h: mask_patterns(h) for h in (0, 1)}
    for c in range(8):
        b, hh = c // 2, c % 2
        qg = qglob_for_core(hh)
        m = dict(common)
        m["xt16"] = xT16[b][:, :NB16]
        m["xt8"] = np.ascontiguousarray(xT8[b][:, NB16:])
        m["xqt16"] = np.ascontiguousarray(xT16[b][:, qg[:SLOT_W]])
        m["xqt8"] = np.ascontiguousarray(xT8[b][:, qg[SLOT_W:]])
        m["mp16"] = mps[hh].astype(NPBF16)
        m["mp8"] = mps[hh].astype(NPF8)
        in_maps.append(m)
    return in_maps


def assemble_out(results):
    out = np.zeros((1, B, S, H), np.float32)
    for c in range(8):
        b, hh = c // 2, c % 2
        qg = qglob_for_core(hh)
        out[0, b, qg, :] = results[c]["out"]
    return out


def kernel(**inputs) -> np.ndarray:
    nc = _get_nc()
    in_maps = make_in_maps(inputs)
    res = run_bass_kernel_spmd(nc, in_maps, list(range(8)))
    return assemble_out(res.results)


# revision 23
# speedup vs baseline: 1.2563x; 1.1786x over previous
"""Causal single-head attention (B=4, S=4096, D=1024, H=128) on 8 NeuronCores.

Sharding: core c = (batch b = c//2, half h = c%2). Each core:
  - computes K^T [h, 4096] and V [4096, H] for its full batch row (replicated
    across the 2 cores of a batch),
  - handles 2048 query rows: 16 parity-interleaved 128-row subtiles
    (global subtile g = 8*r + 2*s + h for slot r in 0..3, s in 0..3),
  - slots have uniform causal k-tile limits [8, 16, 24, 32] so all 8 cores run
    the identical compiled program; causality is enforced with per-core mask
    DATA (host-precomputed 0/1 patterns, shared across slots) multiplied into
    P on the last 8 k-iters of each slot; only the two 128-col subtiles at the
    causal frontier are touched.

Mixed precision (validated vs fp32 reference, rel_err ~3.4e-3):
  - x cols 0:1024 + q-slot-0 columns arrive bf16; everything else fp8 e4m3
    (x pre-scaled x8, weights x64; 1/512 folded into the PSUM->SBUF copy)
  - K^T bricks 0..7, Q^T slot 0, V bricks 0..7 projected in bf16; all other
    projections fp8 DoubleRow (two 128-chunks contracted per matmul, 2x PE)
  - scores always bf16 (fp8 gains nothing at contraction depth 128); P and V
    fp8 for slots 1..3 (q >= 1024): PV runs DoubleRow over k-tile pairs, exp
    is one ACT op per pair over a 2-bank PSUM sT pair; slot 0 stays bf16
Engine balance: odd-kt denominator adds of slots 1,3 and their masks run on
GpSimd (Pool); even adds + slots 0,2 masks on DVE; exp/copies on ACT.
Denominator: dacc (bf16 partial sums per k-lane) contracted against a
sqrt(H)-valued ones vector, [128q,1] matmuls -> fp32 PSUM -> DVE reciprocal.
DMA: constants packed host-side into per-partition-contiguous tensors (2-6KB
descriptors instead of 4-256B), x in 2-chunk transfers, two HW queues (sync,
scalar) fed in mirrored need-time priority order.
"""

import numpy as np
import ml_dtypes
from contextlib import ExitStack

import concourse.bass as bass
import concourse.tile as tile
from concourse import bacc, mybir
from concourse.bass_utils import run_bass_kernel_spmd

B, S, D, H = 4, 4096, 1024, 128
P = 128
BF16 = mybir.dt.bfloat16
F32 = mybir.dt.float32
F8 = mybir.dt.float8e4
NPBF16 = ml_dtypes.bfloat16
NPF8 = ml_dtypes.float8_e4m3

QLOC = 2048          # query rows per core
NSLOT = 4            # slots per core
SLOT_W = 512         # q columns per slot
LIMITS = [8, 16, 24, 32]   # k-tile limit per slot (same for every core)
NKT = S // P         # 32 k tiles
DCH = D // P         # 8 contraction chunks
SX = 8.0             # host scale on x before fp8 cast
SW = 64.0            # host scale on W before fp8 cast (Q,K); V uses SW/8
QKS = 64.0 * SX      # Q/K PSUM arrives as QKS*(xW) on both paths
VS = 64.0            # V PSUM arrives as VS*(xW) on both paths
SCALE = 1.0 / (float(np.sqrt(H)) * QKS * QKS)   # pre-exp scale (+descale)
ONEVAL = VS * float(np.sqrt(H))     # denominator ones: folds sqrt(H) and 1/VS
NB16 = 1024          # x columns (and q rows) kept in bf16
DR = mybir.MatmulPerfMode.DoubleRow
WIX = {"wq": 0, "wk": 1, "wv": 2}
BIX = {"bq": 0, "bk": 1, "bv": 2}


def qglob_for_core(h):
    """Global query row indices (length QLOC) handled by core-half h, in local order."""
    idx = []
    for r in range(NSLOT):
        for s in range(4):
            g = 8 * r + 2 * s + h
            idx.append(np.arange(g * P, (g + 1) * P))
    return np.concatenate(idx)


def mask_patterns(h):
    """[P, 8, 2P] 0/1 causal patterns: entry u covers window columns
    [128*(u//2), 128*(u//2)+256) of a slot at mask k-iter u (kt = 8r+u)."""
    pat = np.zeros((P, 8, 2 * P), np.float32)
    p = np.arange(P)[:, None]
    for u in range(8):
        qq = np.arange(2 * P) + P * (u // 2)
        qpos = (2 * (qq // P) + h) * P + qq % P
        pat[:, u, :] = (qpos[None, :] >= u * P + p)
    return pat


def build_nc():
    nc = bacc.Bacc(None, target_bir_lowering=False, debug=False, num_devices=8)

    xt16 = nc.dram_tensor("xt16", [D, NB16], BF16, kind="ExternalInput").ap()
    xt8 = nc.dram_tensor("xt8", [D, S - NB16], F8, kind="ExternalInput").ap()
    xqt16 = nc.dram_tensor("xqt16", [D, SLOT_W], BF16, kind="ExternalInput").ap()
    xqt8 = nc.dram_tensor("xqt8", [D, QLOC - SLOT_W], F8, kind="ExternalInput").ap()
    wp16 = nc.dram_tensor("wp16", [P, 3, DCH, H], BF16, kind="ExternalInput").ap()
    wp8 = nc.dram_tensor("wp8", [P, 3, DCH, H], F8, kind="ExternalInput").ap()
    bpk = nc.dram_tensor("bpk", [P, 3], F32, kind="ExternalInput").ap()
    idon = nc.dram_tensor("idon", [P, P + 1], BF16, kind="ExternalInput").ap()
    mp16 = nc.dram_tensor("mp16", [P, 8, 2 * P], BF16, kind="ExternalInput").ap()
    mp8 = nc.dram_tensor("mp8", [P, 8, 2 * P], F8, kind="ExternalInput").ap()
    out = nc.dram_tensor("out", [QLOC, H], F32, kind="ExternalOutput").ap()

    Ident = mybir.ActivationFunctionType.Identity
    Copy = mybir.ActivationFunctionType.Copy
    Exp = mybir.ActivationFunctionType.Exp

    with tile.TileContext(nc) as tc, ExitStack() as ctx:
        consts = ctx.enter_context(tc.tile_pool(name="consts", bufs=1))
        persist = ctx.enter_context(tc.tile_pool(name="persist", bufs=1))

        # ---- SBUF destinations (packed)
        wp16_sb = consts.tile([P, 3, DCH, H], BF16, tag="wp16")
        wp8_sb = consts.tile([P, 3, DCH, H], F8, tag="wp8")
        bpk_sb = consts.tile([P, 3], F32, tag="bpk")
        idon_sb = consts.tile([P, P + 1], BF16, tag="idon")
        mp16_sb = consts.tile([P, 8, 2 * P], BF16, tag="mp16")
        mp8_sb = consts.tile([P, 8, 2 * P], F8, tag="mp8")
        identb_v = idon_sb[:, 0:P]
        ones_v = idon_sb[:, P:P + 1]

        def bias(nm):
            return bpk_sb[:, BIX[nm]:BIX[nm] + 1]

        kT = persist.tile([P, S], BF16, tag="kT")             # K^T [h, s]
        vN16 = persist.tile([P, 8, H], BF16, tag="vN16")      # V natural, bricks 0..7
        vN8 = persist.tile([P, NKT, H], F8, tag="vN8")        # V natural fp8, all bricks
        qT = persist.tile([P, QLOC], BF16, tag="qT")          # Q^T [h, q_local]
        xt16_sb = persist.tile([P, DCH, NB16], BF16, tag="xt16_sb")
        xt8_sb = persist.tile([P, DCH, S - NB16], F8, tag="xt8_sb")
        xqt16_sb = persist.tile([P, DCH, SLOT_W], BF16, tag="xqt16_sb")
        xqt8_sb = persist.tile([P, DCH, QLOC - SLOT_W], F8, tag="xqt8_sb")

        # ---- DMA: chunk-pair transfers, two queues in mirrored priority order
        def load_pair(eng, dst, src, jp):
            eng.dma_start(
                out=dst[:, 2 * jp:2 * jp + 2, :],
                in_=src[2 * jp * P:(2 * jp + 2) * P, :].rearrange(
                    "(c p) s -> p c s", p=P),
            )

        # group 1: first K/V projection critical path
        nc.sync.dma_start(out=bpk_sb[:], in_=bpk)
        nc.sync.dma_start(out=wp16_sb[:], in_=wp16)
        load_pair(nc.sync, xt16_sb, xt16, 0)
        load_pair(nc.scalar, xt16_sb, xt16, 2)
        load_pair(nc.sync, xt16_sb, xt16, 1)
        load_pair(nc.scalar, xt16_sb, xt16, 3)
        # group 2: Q slot 0 + V transpose ident + early (slot-0) mask patterns
        nc.sync.dma_start(out=idon_sb[:], in_=idon)
        load_pair(nc.scalar, xqt16_sb, xqt16, 0)
        load_pair(nc.scalar, xqt16_sb, xqt16, 1)
        nc.sync.dma_start(out=mp16_sb[:], in_=mp16)
        load_pair(nc.scalar, xqt16_sb, xqt16, 2)
        load_pair(nc.scalar, xqt16_sb, xqt16, 3)
        # group 3: fp8 stripes 2,3 (kT/vN bricks 8..15, pass-1 tail)
        nc.sync.dma_start(out=wp8_sb[:], in_=wp8)
        load_pair(nc.sync, xt8_sb, xt8, 0)
        load_pair(nc.scalar, xt8_sb, xt8, 1)
        nc.sync.dma_start(out=mp8_sb[:], in_=mp8)
        # group 4: Q slots 1..3
        load_pair(nc.sync, xqt8_sb, xqt8, 0)
        load_pair(nc.scalar, xqt8_sb, xqt8, 2)
        load_pair(nc.sync, xqt8_sb, xqt8, 1)
        load_pair(nc.scalar, xqt8_sb, xqt8, 3)
        # group 5: fp8 stripes 4..7 (pass 2)
        load_pair(nc.sync, xt8_sb, xt8, 2)
        load_pair(nc.scalar, xt8_sb, xt8, 3)

        # PSUM budget (8 banks): mm x2 + sT-pair x2(4 banks) + oT-pair (2) = 8
        with tc.tile_pool(name="stg", bufs=3) as stg, \
             tc.tile_pool(name="mm", bufs=2, space="PSUM") as psA, \
             tc.tile_pool(name="psS", bufs=2, space="PSUM") as psS, \
             tc.tile_pool(name="psO", bufs=1, space="PSUM") as psO, \
             tc.tile_pool(name="pp", bufs=4) as pp, \
             tc.tile_pool(name="pp8", bufs=4) as pp8, \
             tc.tile_pool(name="acc", bufs=1) as accp, \
             tc.tile_pool(name="epi", bufs=3) as epi:

            def project16(src_sb, srs, wname):
                """bf16 weight-stationary projection of 512-col stripes of src."""
                pss = [psA.tile([P, SLOT_W], F32, tag="mm512", name=f"p16_{i}")
                       for i in range(len(srs))]
                for j in range(DCH):
                    for i, sr in enumerate(srs):
                        nc.tensor.matmul(
                            pss[i][:], lhsT=wp16_sb[:, WIX[wname], j, :],
                            rhs=src_sb[:, j, sr * SLOT_W:(sr + 1) * SLOT_W],
                            start=(j == 0), stop=(j == DCH - 1),
                        )
                return pss

            def project8(src_sb, srs, wname):
                """fp8 DoubleRow projection (chunk pairs) of 512-col stripes."""
                pss = [psA.tile([P, SLOT_W], F32, tag="mm512", name=f"p8_{i}")
                       for i in range(len(srs))]
                for j in range(DCH // 2):
                    for i, sr in enumerate(srs):
                        nc.tensor.matmul(
                            pss[i][:], lhsT=wp8_sb[:, WIX[wname], 2 * j:2 * j + 2, :],
                            rhs=src_sb[:, 2 * j:2 * j + 2, sr * SLOT_W:(sr + 1) * SLOT_W],
                            start=(j == 0), stop=(j == DCH // 2 - 1), perf_mode=DR,
                        )
                return pss

            def transpose_bricks(vTs, sr, dst):
                """PE-transpose a bf16 512-col V^T stripe into 4 natural bricks
                of dst (fp8 dst converts on the DVE copy)."""
                pst = psA.tile([P, SLOT_W], BF16, tag="mm512", name="vtr")
                for t_ in range(4):
                    nc.tensor.matmul(
                        pst[:, t_ * P:(t_ + 1) * P], lhsT=vTs[:, t_ * P:(t_ + 1) * P],
                        rhs=identb_v, is_transpose=True, skip_group_check=True,
                    )
                nc.vector.tensor_copy(dst[:, sr * 4:(sr + 1) * 4, :], pst[:])

            def kv_common(kps, vps, srs):
                """PSUM->SBUF: K^T via DVE bias-add (frees ACT for exp);
                V^T via ACT bias-add, then PE transpose."""
                for ps, sr in zip(kps, srs):
                    nc.vector.tensor_scalar_add(
                        kT[:, sr * SLOT_W:(sr + 1) * SLOT_W], ps[:], bias("bk"))
                for ps, sr in zip(vps, srs):
                    vTs = stg.tile([P, SLOT_W], BF16, tag="vT")
                    nc.scalar.activation(vTs[:], ps[:], Ident, bias=bias("bv"),
                                         scale=1.0)
                    transpose_bricks(vTs, sr, vN16 if sr < 2 else vN8)

            def kv_stripes16():
                kv_common(project16(xt16_sb, (0, 1), "wk"),
                          project16(xt16_sb, (0, 1), "wv"), (0, 1))
                nc.vector.tensor_copy(vN8[:, 0:8, :], vN16[:])

            def kv_stripes8(*srs):
                lsrs = [sr - 2 for sr in srs]   # xt8 local stripe index
                kv_common(project8(xt8_sb, lsrs, "wk"),
                          project8(xt8_sb, lsrs, "wv"), srs)

            def q_slot0():
                (ps,) = project16(xqt16_sb, (0,), "wq")
                nc.vector.tensor_scalar_add(qT[:, 0:SLOT_W], ps[:], bias("bq"))

            def q_slots8(*qrs):
                lqs = [qr - 1 for qr in qrs]   # xqt8 local stripe index
                for ps, qr in zip(project8(xqt8_sb, lqs, "wq"), qrs):
                    nc.vector.tensor_scalar_add(
                        qT[:, qr * SLOT_W:(qr + 1) * SLOT_W], ps[:], bias("bq"))

            def attention_pass(slots):
                """kt-pair-outer attention over a pair of slots (shared K/V)."""
                Ls = {r: LIMITS[r] for r in slots}
                Tmax = max(Ls.values()) // 2
                oT = psO.tile([P, len(slots), SLOT_W], F32, tag="oT")
                dacc = {r: accp.tile([P, SLOT_W], BF16, tag=f"dacc{r}", name=f"dacc{r}")
                        for r in slots}

                def c0_of(r, kt):
                    # first column (h-safe) any core's subtile can still attend
                    # at this k-brick; earlier columns are masked for both
                    # halves and skipped (the pair uses the even-kt value; the
                    # mask pattern zeroes the stale subtile)
                    return P * max(0, (kt - 8 * r) // 2)

                def dacc_upd(r, src_ap, c0, first, pool=False):
                    if first:
                        nc.vector.tensor_copy(dacc[r][:], src_ap)
                    else:
                        eng = nc.gpsimd if pool else nc.vector
                        eng.tensor_add(dacc[r][:, c0:], dacc[r][:, c0:], src_ap)

                def mask_op(r, kt, dst_ap, c0):
                    # p *= 0/1 pattern; only the two subtiles at the causal
                    # frontier are affected. Pool engine for slots 1,3.
                    u = kt - 8 * r
                    w = min(2 * P, SLOT_W - c0)
                    mp = mp16_sb if r == 0 else mp8_sb
                    eng = nc.gpsimd if r in (1, 3) else nc.vector
                    eng.tensor_mul(dst_ap[:, :w], dst_ap[:, :w], mp[:, u, :w])

                def score_pair(t):
                    outs = {}
                    for r in slots:
                        if 2 * t >= Ls[r]:
                            continue
                        c0 = c0_of(r, 2 * t)
                        qsl = slice(r * SLOT_W + c0, (r + 1) * SLOT_W)
                        sTp = psS.tile([P, 2, SLOT_W], F32, tag="sT")
                        for i in range(2):
                            nc.tensor.matmul(
                                sTp[:, i, c0:],
                                lhsT=kT[:, (2 * t + i) * P:(2 * t + i + 1) * P],
                                rhs=qT[:, qsl], start=True, stop=True,
                            )
                        masked = 2 * t >= Ls[r] - 8
                        if r == 0:
                            pTs = []
                            for i in range(2):
                                pT = pp.tile([P, SLOT_W], BF16, tag="pT")
                                nc.scalar.activation(pT[:, c0:], sTp[:, i, c0:],
                                                     Exp, scale=SCALE)
                                if masked:
                                    mask_op(r, 2 * t + i, pT[:, c0:], c0)
                                dacc_upd(r, pT[:, c0:], c0, first=(t == 0 and i == 0))
                                pTs.append(pT)
                            outs[r] = (pTs, c0)
                        else:
                            pT8 = pp8.tile([P, 2, SLOT_W], F8, tag="pT8")
                            nc.scalar.activation(pT8[:, :, c0:], sTp[:, :, c0:],
                                                 Exp, scale=SCALE)
                            for i in range(2):
                                if masked:
                                    mask_op(r, 2 * t + i, pT8[:, i, c0:], c0)
                                dacc_upd(r, pT8[:, i, c0:], c0,
                                         first=(t == 0 and i == 0),
                                         pool=(i == 1 and r in (1, 3)))
                            outs[r] = (pT8, c0)
                    return outs

                def accum_pair(t, outs):
                    for i_s, r in enumerate(slots):
                        if r not in outs:
                            continue
                        buf, c0 = outs[r]
                        if r == 0:
                            for i in range(2):
                                nc.tensor.matmul(
                                    oT[:, i_s, c0:], lhsT=vN16[:, 2 * t + i, :],
                                    rhs=buf[i][:, c0:],
                                    start=(t == 0 and i == 0),
                                    stop=(2 * t + i == Ls[r] - 1),
                                )
                        else:
                            nc.tensor.matmul(
                                oT[:, i_s, c0:], lhsT=vN8[:, 2 * t:2 * t + 2, :],
                                rhs=buf[:, :, c0:],
                                start=(t == 0), stop=(2 * t + 1 == Ls[r] - 1),
                                perf_mode=DR,
                            )

                def epilogue(i_s, r):
                    """O = transpose(O^T) / (d * sqrt(H)) for one slot."""
                    d_ps = psA.tile([P, 4], F32, tag="mm512", name="dmm")
                    for s_ in range(4):
                        nc.tensor.matmul(
                            d_ps[:, s_:s_ + 1],
                            lhsT=dacc[r][:, s_ * P:(s_ + 1) * P], rhs=ones_v,
                            start=(s_ == 0), stop=(s_ == 3), skip_group_check=True,
                        )
                    oTs = epi.tile([P, SLOT_W], BF16, tag="oTs")
                    nc.scalar.activation(oTs[:], oT[:, i_s, :], Copy)
                    rec = epi.tile([P, 4], F32, tag="rec")
                    nc.vector.reciprocal(rec[:], d_ps[:])
                    obr = psA.tile([P, SLOT_W], BF16, tag="mm512", name="obr")
                    for s_ in range(4):
                        nc.tensor.matmul(
                            obr[:, s_ * P:(s_ + 1) * P], lhsT=oTs[:, s_ * P:(s_ + 1) * P],
                            rhs=identb_v, is_transpose=True, skip_group_check=True,
                        )
                    ofin = epi.tile([P, SLOT_W], F32, tag="ofin")
                    for s_ in range(4):
                        nc.vector.tensor_scalar_mul(
                            ofin[:, s_ * P:(s_ + 1) * P], obr[:, s_ * P:(s_ + 1) * P],
                            rec[:, s_:s_ + 1],
                        )
                    nc.sync.dma_start(
                        out=out[r * SLOT_W:(r + 1) * SLOT_W, :].rearrange(
                            "(s p) h -> p s h", p=P
                        ),
                        in_=ofin[:].rearrange("p (s h) -> p s h", s=4),
                    )

                prev = score_pair(0)
                for t in range(1, Tmax):
                    cur = score_pair(t)
                    accum_pair(t - 1, prev)
                    prev = cur
                    # emit the shorter slot's epilogue as soon as it stops
                    for i_s, r in enumerate(slots):
                        if Ls[r] == 2 * t:
                            epilogue(i_s, r)
                accum_pair(Tmax - 1, prev)
                for i_s, r in enumerate(slots):
                    if Ls[r] == 2 * Tmax:
                        epilogue(i_s, r)

            # emission: all projections first (their PSUM slot allocations must
            # not queue behind pass epilogues), then the attention passes
            kv_stripes16()
            q_slot0()
            kv_stripes8(2, 3)
            q_slots8(1)
            kv_stripes8(4, 5)
            kv_stripes8(6, 7)
            q_slots8(2, 3)
            attention_pass((0, 1))
            attention_pass((2, 3))

    nc.compile()
    return nc


_NC_CACHE = None


def _get_nc():
    global _NC_CACHE
    if _NC_CACHE is None:
        _NC_CACHE = build_nc()
    return _NC_CACHE


def make_in_maps(inputs):
    x = np.asarray(inputs["x"], np.float32)
    Ws = {nm: np.asarray(inputs[Wnm], np.float32)
          for nm, Wnm in (("wq", "Wq"), ("wk", "Wk"), ("wv", "Wv"))}
    bs = {nm: np.asarray(inputs[bnm], np.float32)
          for nm, bnm in (("bq", "bq"), ("bk", "bk"), ("bv", "bv"))}

    # packed weights: wp[p, i, c, h] = W_i[c*128+p, h]
    wp16 = np.zeros((P, 3, DCH, H), NPBF16)
    wp8 = np.zeros((P, 3, DCH, H), NPF8)
    for nm, W in Ws.items():
        s16 = QKS if nm in ("wq", "wk") else VS
        Wc = W.reshape(DCH, P, H).transpose(1, 0, 2)   # [p, c, h]
        wp16[:, WIX[nm]] = (Wc * s16).astype(NPBF16)
        wp8[:, WIX[nm]] = (Wc * (s16 / SX)).astype(NPF8)
    bpk = np.zeros((P, 3), np.float32)
    for nm, b in bs.items():
        bpk[:, BIX[nm]] = b * (QKS if nm in ("bq", "bk") else VS)
    idon = np.concatenate(
        [np.eye(P, dtype=NPBF16), np.full((P, 1), ONEVAL, NPBF16)], axis=1)

    common = dict(wp16=wp16, wp8=wp8, bpk=bpk, idon=idon)
    in_maps = []
    xT = np.ascontiguousarray(x.transpose(0, 2, 1))        # [B, D, S] fp32
    xT16 = xT.astype(NPBF16)
    xT8 = (xT * SX).astype(NPF8)
    mps = {h: mask_patterns(h) for h in (0, 1)}
    for c in range(8):
        b, hh = c // 2, c % 2
        qg = qglob_for_core(hh)
        m = dict(common)
        m["xt16"] = xT16[b][:, :NB16]
        m["xt8"] = np.ascontiguousarray(xT8[b][:, NB16:])
        m["xqt16"] = np.ascontiguousarray(xT16[b][:, qg[:SLOT_W]])
        m["xqt8"] = np.ascontiguousarray(xT8[b][:, qg[SLOT_W:]])
        m["mp16"] = mps[hh].astype(NPBF16)
        m["mp8"] = mps[hh].astype(NPF8)
        in_maps.append(m)
    return in_maps


def assemble_out(results):
    out = np.zeros((1, B, S, H), np.float32)
    for c in range(8):
        b, hh = c // 2, c % 2
        qg = qglob_for_core(hh)
        out[0, b, qg, :] = results[c]["out"]
    return out


def kernel(**inputs) -> np.ndarray:
    nc = _get_nc()
    in_maps = make_in_maps(inputs)
    res = run_bass_kernel_spmd(nc, in_maps, list(range(8)))
    return assemble_out(res.results)
